# revision 45
# baseline (speedup 1.0000x reference)
"""Trainium2 Bass kernel for causal GQA flash-attention + KV-cache store.

Problem shapes (hardcoded):
    q: [4096, 32, 128] f32, k/v: [4096, 8, 128] f32, caches [4096, 1024] f32,
    slot_mapping arange(4096) int32, seq_len 2048 -> B=2 sequences.

Sharding: 8 cores; core c owns KV head c (4 query heads) for both sequences
(2 (b, kv) "units" per core). Data-parallel over batch x tensor-parallel over
KV heads; KV cache sharded by head. Host does layout transforms (transpose,
fp16 cast) as part of sharding; the device kernel does all FLOPs and the
cache-update copies.

Per-unit device pipeline, blocked over 16 s-blocks (128 s-positions x 4 heads
= 512 "sq" columns) and causal t-tiles of 128:
    matmul1: S^T[t,sq] = K^T[d,t].T @ Q^T[d,sq]   (fp16, f32 PSUM)
    ACT:     P^T = exp(SCALE * S^T)  PSUM->SBUF fp16
    DVE:     diagonal t-tile multiplied by causal 0/1 mask
    matmul2: O[sq,129] += P^T[:,chunk].T @ [V | 1] (ones column makes col 128
             the softmax denominator)
    DVE:     reciprocal + per-partition scale, DMA out.
"""

import math

import numpy as np

import concourse.bass as bass
import concourse.mybir as mybir
import concourse.tile as tile
from concourse import bacc
from concourse.bass_utils import run_bass_kernel_spmd

B, S, H, HKV, G, D = 2, 2048, 32, 8, 4, 128
N = B * S
NBLK = S // 128  # 16 s-blocks per sequence
VCOLS = 256  # V columns padded: d(128) | ones(1) | zeros(127)
SCALE = 1.0 / math.sqrt(D)
NCORES = 8

FP16 = mybir.dt.float16
F32 = mybir.dt.float32

_NC = None


def _build_nc():
    nc = bacc.Bacc("TRN2", target_bir_lowering=False)

    qt = nc.dram_tensor("qt", [B, 128, NBLK * 512], FP16, kind="ExternalInput")
    kt = nc.dram_tensor("kt", [B, 128, S], FP16, kind="ExternalInput")
    va = nc.dram_tensor("va", [B, 128, NBLK * VCOLS], FP16, kind="ExternalInput")
    knat = nc.dram_tensor("knat", [B, S, D], F32, kind="ExternalInput")
    vnat = nc.dram_tensor("vnat", [B, S, D], F32, kind="ExternalInput")
    # Multiplicative 0/1 causal mask for the diagonal t-tile, applied on the
    # (underutilized) vector engine after exp: mk[t_in, (g, s_in)] = t_in<=s_in.
    mk = nc.dram_tensor("mk", [128, 512], FP16, kind="ExternalInput")

    # Out layout [u, s_in, blk, (g,d)]: a 4-block batched store then has
    # 8KB-contiguous per-partition runs (big DMA descriptors), instead of
    # 2KB with a blk-major layout. Host untangles it.
    o = nc.dram_tensor("o", [B, 128, NBLK, 512], F32, kind="ExternalOutput")
    st = nc.dram_tensor("st", [128, 896], FP16, kind="ExternalInput")
    kc = nc.dram_tensor("kc", [B, S, D], F32, kind="ExternalOutput")
    vc = nc.dram_tensor("vc", [B, S, D], F32, kind="ExternalOutput")

    with tile.TileContext(nc) as tc:
        with (
            tc.tile_pool(name="consts", bufs=1) as consts,
            tc.tile_pool(name="big", bufs=2) as big,
            tc.tile_pool(name="pt", bufs=6) as ptp,
            tc.tile_pool(name="osb", bufs=6) as obp,
            tc.tile_pool(name="nrm", bufs=4) as nrm,
            tc.tile_pool(name="st", bufs=2, space="PSUM") as stp,
            tc.tile_pool(name="op", bufs=2, space="PSUM") as opp,
        ):
            # Mask on the (otherwise idle) SWDGE ring so it lands in
            # parallel with the sync-ring input stream.
            mk_t = consts.tile([128, 512], FP16, tag="mk")
            nc.gpsimd.dma_start(out=mk_t, in_=mk[:, :])

            # All input loads for BOTH units plus the cache copies are
            # emitted up front on the sync ring: none of them have compute
            # dependencies, so the SP sequencer streams them back to back
            # from t=0. Output stores are emitted later (also on sync) and
            # their data-waits resolve just in time.
            kt_cs, qt_cs, va_cs = [], [], []
            starters = {}
            for u in range(B):
                kt_c = [None, None]  # [128,1024]: t-tiles 8i..8i+7
                qt_c = [None] * 4  # [128,2048]: blocks 4i..4i+3
                va_c = [None, None]  # [128,2048]: t-tiles 8i..8i+7
                kt_cs.append(kt_c)
                qt_cs.append(qt_c)
                va_cs.append(va_c)

                def _load(kind, i, u=u, kt_c=kt_c, qt_c=qt_c, va_c=va_c):
                    if kind == "kt":
                        t = big.tile(
                            [128, 1024], FP16, tag=f"kt{i}", name=f"kt{u}_{i}"
                        )
                        nc.sync.dma_start(
                            out=t, in_=kt[u, :, i * 1024 : (i + 1) * 1024]
                        )
                        kt_c[i] = t
                    elif kind == "qt":
                        t = big.tile(
                            [128, 2048], FP16, tag=f"qt{i}", name=f"qt{u}_{i}"
                        )
                        nc.sync.dma_start(
                            out=t, in_=qt[u, :, i * 2048 : (i + 1) * 2048]
                        )
                        qt_c[i] = t
                    else:
                        t = big.tile(
                            [128, 8 * VCOLS], FP16, tag=f"va{i}", name=f"va{u}_{i}"
                        )
                        nc.sync.dma_start(
                            out=t, in_=va[u, :, i * 8 * VCOLS : (i + 1) * 8 * VCOLS]
                        )
                        va_c[i] = t

                if u == 0:
                    # One packed starter tile covers block 0 (t-tile 0 only)
                    # so the first matmul issues a few microseconds in.
                    st_t = big.tile([128, 896], FP16, tag="st", name="st_t")
                    nc.scalar.dma_start(out=st_t, in_=st[:, :])
                    starters[0] = (
                        st_t[:, 0:128], st_t[:, 128:640], st_t[:, 640:896],
                    )
                    order = [
                        ("kt", 0), ("qt", 0), ("va", 0), ("qt", 1),
                        ("va", 1), ("qt", 2), ("kt", 1), ("qt", 3),
                    ]
                else:
                    # Unit 1 runs its blocks in descending order, so the
                    # tail-needed tiles load first .. (everything is resident
                    # long before unit 1 starts anyway).
                    order = [
                        ("qt", 3), ("kt", 1), ("va", 1), ("va", 0),
                        ("kt", 0), ("qt", 2), ("qt", 1), ("qt", 0),
                    ]
                for kind, i in order:
                    _load(kind, i)

            # KV-cache store (slot_mapping is arange -> token-order copy);
            # DRAM->DRAM right behind the input loads, well before the
            # output stores need the ring.
            for u in range(B):
                nc.sync.dma_start(out=kc[u], in_=knat[u])
                nc.sync.dma_start(out=vc[u], in_=vnat[u])

            # One group = up to 2 t-tiles sharing an exp (FD=1024). The PV
            # matmuls for group g are emitted AFTER group g+1's QK matmuls
            # (lag-1 software pipeline) so the TensorE never sits behind the
            # exp on the critical path.
            pending = [None]  # (blk_done, o_ps, p_t, blk, t0, gsz, n_t, va_c)
            o_sb_cur = [None, 0]  # current 4-block out staging tile, #filled

            def _emit_mm2_and_maybe_norm(u):
                if pending[0] is None:
                    return
                (is_last, o_ps, p_t, blk, t0, gsz, n_t, rhs_list) = pending[0]
                pending[0] = None
                for j in range(gsz):
                    tt = t0 + j
                    for c in range(4):
                        # Two accumulation chunks share a PSUM bank; only the
                        # first may use start=True (start clears has_written
                        # bits of the WHOLE bank). The second chunk's first
                        # write lands on cleared bits and overwrites, so
                        # start=False always is correct.
                        nc.tensor.matmul(
                            o_ps[c // 2][:, (c % 2) * 129 : (c % 2) * 129 + 129],
                            lhsT=p_t[
                                :, j * 512 + c * 128 : j * 512 + (c + 1) * 128
                            ],
                            rhs=rhs_list[j],
                            start=(tt == 0 and c % 2 == 0),
                            stop=(tt == n_t - 1),
                            skip_group_check=(c % 2 == 1),
                        )
                if not is_last:
                    return
                sums = nrm.tile([128, 4], F32, tag="sums", name="sums")
                rec = nrm.tile([128, 4], F32, tag="rec", name="rec")
                for i in range(2):
                    src = o_ps[i].rearrange("p (a b) -> p a b", a=2)[:, :, 128]
                    nc.vector.tensor_copy(sums[:, 2 * i : 2 * i + 2], src)
                nc.vector.reciprocal(rec, sums)
                if u == 1 and blk < 4:
                    # The kernel's last four (small) blocks: store each
                    # immediately so the final transfer trailing the last
                    # compute is 256KB, not a 1MB batch.
                    o1 = obp.tile([128, 512], F32, tag="osb1", name="o_sb1")
                    for c in range(4):
                        nc.vector.tensor_scalar_mul(
                            o1[:, c * 128 : (c + 1) * 128],
                            o_ps[c // 2][:, (c % 2) * 129 : (c % 2) * 129 + 128],
                            rec[:, c : c + 1],
                        )
                    nc.gpsimd.dma_start(
                        out=o[u, :, blk : blk + 1, :],
                        in_=o1.rearrange("p (k f) -> p k f", k=1),
                    )
                    return
                # Normalized blocks stage into a 4-block batch tile; one DMA
                # per batch gives 8KB descriptors and fewer SWDGE ops.
                if o_sb_cur[0] is None:
                    o_sb_cur[0] = obp.tile([128, 2048], F32, tag="osb", name="o_sb")
                    o_sb_cur[1] = 0
                o_sb = o_sb_cur[0]
                off = (blk % 4) * 512
                for c in range(4):
                    nc.vector.tensor_scalar_mul(
                        o_sb[:, off + c * 128 : off + (c + 1) * 128],
                        o_ps[c // 2][:, (c % 2) * 129 : (c % 2) * 129 + 128],
                        rec[:, c : c + 1],
                    )
                o_sb_cur[1] += 1
                if o_sb_cur[1] == 4:
                    b0 = (blk // 4) * 4
                    nc.gpsimd.dma_start(
                        out=o[u, :, b0 : b0 + 4, :],
                        in_=o_sb.rearrange("p (k f) -> p k f", k=4),
                    )
                    o_sb_cur[0] = None

            for u in range(B):
                kt_c, qt_c, va_c = kt_cs[u], qt_cs[u], va_cs[u]
                if u in starters:
                    kt_st, qt_st, va_st = starters[u]

                blk_order = range(NBLK) if u == 0 else reversed(range(NBLK))
                for blk in blk_order:
                    n_t = blk + 1
                    starter = u == 0 and blk == 0
                    if starter:
                        rhs_q = qt_st
                    else:
                        rhs_q = qt_c[blk // 4][
                            :, (blk % 4) * 512 : (blk % 4 + 1) * 512
                        ]
                    o_ps = [
                        opp.tile([128, 258], F32, tag=f"o{i}", name=f"o_ps{i}")
                        for i in range(2)
                    ]
                    # Pair t-tiles for FD=1024 exps, but ALWAYS end the block
                    # with a single-tile group: the short final exp releases
                    # the cross-block PSUM WAR dependency ~520ns sooner, which
                    # is the recurring PE bubble at block boundaries.
                    if n_t % 2 == 0:
                        groups = (
                            [(0, 1)]
                            + [(t, 2) for t in range(1, n_t - 1, 2)]
                            + [(n_t - 1, 1)]
                        )
                    else:
                        groups = [(t, 2) for t in range(0, n_t - 1, 2)] + [
                            (n_t - 1, 1)
                        ]
                    for t0, gsz in groups:
                        s_ps = stp.tile([128, 1024], F32, tag="s", name="s_ps")
                        for j in range(gsz):
                            tt = t0 + j
                            sl = s_ps[:, j * 512 : (j + 1) * 512]
                            nc.tensor.matmul(
                                sl,
                                lhsT=(
                                    kt_st
                                    if starter
                                    else kt_c[tt // 8][
                                        :, (tt % 8) * 128 : (tt % 8 + 1) * 128
                                    ]
                                ),
                                rhs=rhs_q,
                                start=True,
                                stop=True,
                            )
                        p_t = ptp.tile([128, 1024], FP16, tag="p", name="p_t")
                        nc.scalar.activation(
                            p_t[:, : gsz * 512],
                            s_ps[:, : gsz * 512],
                            mybir.ActivationFunctionType.Exp,
                            scale=SCALE,
                        )
                        if t0 + gsz == n_t:
                            # Diagonal t-tile (always the last of the block):
                            # zero the upper-triangular part after exp.
                            dsl = p_t[:, (gsz - 1) * 512 : gsz * 512]
                            nc.vector.tensor_mul(dsl, dsl, mk_t)
                        rhs_list = [
                            (
                                va_st[:, 0:129]
                                if starter
                                else va_c[(t0 + j) // 8][
                                    :,
                                    ((t0 + j) % 8) * VCOLS : ((t0 + j) % 8) * VCOLS
                                    + 129,
                                ]
                            )
                            for j in range(gsz)
                        ]
                        _emit_mm2_and_maybe_norm(u)
                        pending[0] = (
                            t0 + gsz == n_t, o_ps, p_t, blk, t0, gsz, n_t, rhs_list,
                        )
                _emit_mm2_and_maybe_norm(u)

    nc.compile()
    return nc


def _get_nc():
    global _NC
    if _NC is None:
        _NC = _build_nc()
    return _NC


def _prepare_in_maps(q, k, v):
    q = np.asarray(q, dtype=np.float32)
    k = np.asarray(k, dtype=np.float32)
    v = np.asarray(v, dtype=np.float32)

    # qt[kv, b, d, blk, g, s_in]
    q6 = q.reshape(B, NBLK, 128, HKV, G, D)
    qt = np.ascontiguousarray(
        np.transpose(q6, (3, 0, 5, 1, 4, 2)).astype(np.float16)
    ).reshape(HKV, B, 128, NBLK * 512)

    # kt[kv, b, d, t]
    k4 = k.reshape(B, S, HKV, D)
    kt = np.ascontiguousarray(np.transpose(k4, (2, 0, 3, 1)).astype(np.float16))

    # va[kv, b, t_in, tt, col]: col<128 = v, col 128 = 1.0, rest 0
    v5 = v.reshape(B, NBLK, 128, HKV, D)
    va = np.zeros((HKV, B, 128, NBLK, VCOLS), dtype=np.float16)
    va[..., :D] = np.transpose(v5, (3, 0, 2, 1, 4))
    va[..., D] = 1.0
    va = va.reshape(HKV, B, 128, NBLK * VCOLS)

    knat = np.ascontiguousarray(np.transpose(k4, (2, 0, 1, 3)))
    vnat = np.ascontiguousarray(
        np.transpose(v.reshape(B, S, HKV, D), (2, 0, 1, 3))
    )

    idx = np.arange(128)
    mk = np.tile(
        (idx[:, None] <= idx[None, :]).astype(np.float16), (1, G)
    )  # [t_in, (g, s_in)] = t_in <= s_in

    in_maps = []
    for c in range(NCORES):
        starter = np.concatenate(
            [kt[c, 0, :, 0:128], qt[c, 0, :, 0:512], va[c, 0, :, 0:VCOLS]], axis=1
        )
        in_maps.append(
            {
                "qt": qt[c],
                "kt": kt[c],
                "va": va[c],
                "knat": knat[c],
                "vnat": vnat[c],
                "mk": mk,
                "st": np.ascontiguousarray(starter),
            }
        )
    return in_maps


def _assemble(results, k_cache, v_cache, slot_mapping):
    o_dev = np.stack([results[c]["o"] for c in range(NCORES)])
    o_dev = o_dev.reshape(HKV, B, 128, NBLK, G, D)  # [kv, b, s_in, blk, g, d]
    out = np.ascontiguousarray(
        np.transpose(o_dev, (1, 3, 2, 0, 4, 5))
    ).reshape(N, H, D)

    slots = np.asarray(slot_mapping)
    kcf = np.array(np.asarray(k_cache), dtype=np.float32, copy=True)
    vcf = np.array(np.asarray(v_cache), dtype=np.float32, copy=True)
    kc_dev = np.stack([results[c]["kc"] for c in range(NCORES)])  # [kv, b, t, d]
    vc_dev = np.stack([results[c]["vc"] for c in range(NCORES)])
    kc_rows = np.transpose(kc_dev, (1, 2, 0, 3)).reshape(N, HKV * D)
    vc_rows = np.transpose(vc_dev, (1, 2, 0, 3)).reshape(N, HKV * D)
    kcf[slots] = kc_rows
    vcf[slots] = vc_rows
    return out, kcf, vcf


def _run(q, k, v, k_cache, v_cache, slot_mapping, seq_len, trace=False):
    assert int(seq_len) == S
    nc = _get_nc()
    in_maps = _prepare_in_maps(q, k, v)
    res = run_bass_kernel_spmd(nc, in_maps, core_ids=list(range(NCORES)), trace=trace)
    out, kcf, vcf = _assemble(res.results, k_cache, v_cache, slot_mapping)
    return (out, kcf, vcf), res


def kernel(q, k, v, k_cache, v_cache, slot_mapping, seq_len):
    (out, kcf, vcf), _ = _run(q, k, v, k_cache, v_cache, slot_mapping, seq_len)
    return out, kcf, vcf


# revision 47
# speedup vs baseline: 1.0364x; 1.0364x over previous
"""Trainium2 Bass kernel for causal GQA flash-attention + KV-cache store.

Problem shapes (hardcoded):
    q: [4096, 32, 128] f32, k/v: [4096, 8, 128] f32, caches [4096, 1024] f32,
    slot_mapping arange(4096) int32, seq_len 2048 -> B=2 sequences.

Sharding: 8 cores; core c owns KV head c (4 query heads) for both sequences
(2 (b, kv) "units" per core). Data-parallel over batch x tensor-parallel over
KV heads; KV cache sharded by head. Host does layout transforms (transpose,
fp16 cast) as part of sharding; the device kernel does all FLOPs and the
cache-update copies.

Per-unit device pipeline, blocked over 16 s-blocks (128 s-positions x 4 heads
= 512 "sq" columns) and causal t-tiles of 128:
    matmul1: S^T[t,sq] = K^T[d,t].T @ Q^T[d,sq]   (fp16, f32 PSUM)
    ACT:     P^T = exp(SCALE * S^T)  PSUM->SBUF fp16
    DVE:     diagonal t-tile multiplied by causal 0/1 mask
    matmul2: O[sq,129] += P^T[:,chunk].T @ [V | 1] (ones column makes col 128
             the softmax denominator)
    DVE:     reciprocal + per-partition scale, DMA out.
"""

import math

import numpy as np

import concourse.bass as bass
import concourse.mybir as mybir
import concourse.tile as tile
from concourse import bacc
from concourse.bass_utils import run_bass_kernel_spmd

B, S, H, HKV, G, D = 2, 2048, 32, 8, 4, 128
N = B * S
NBLK = S // 128  # 16 s-blocks per sequence
VCOLS = 256  # V columns padded: d(128) | ones(1) | zeros(127)
SCALE = 1.0 / math.sqrt(D)
NCORES = 8

FP16 = mybir.dt.float16
F32 = mybir.dt.float32

_NC = None


def _build_nc():
    nc = bacc.Bacc("TRN2", target_bir_lowering=False)

    qt = nc.dram_tensor("qt", [B, 128, NBLK * 512], FP16, kind="ExternalInput")
    kt = nc.dram_tensor("kt", [B, 128, S], FP16, kind="ExternalInput")
    va = nc.dram_tensor("va", [B, 128, NBLK * VCOLS], FP16, kind="ExternalInput")
    knat = nc.dram_tensor("knat", [B, S, D], F32, kind="ExternalInput")
    vnat = nc.dram_tensor("vnat", [B, S, D], F32, kind="ExternalInput")
    # Multiplicative 0/1 causal mask for the diagonal t-tile, applied on the
    # (underutilized) vector engine after exp: mk[t_in, (g, s_in)] = t_in<=s_in.
    mk = nc.dram_tensor("mk", [128, 512], FP16, kind="ExternalInput")

    # Out layout [u, s_in, blk, (g,d)]: a 4-block batched store then has
    # 8KB-contiguous per-partition runs (big DMA descriptors), instead of
    # 2KB with a blk-major layout. Host untangles it.
    o = nc.dram_tensor("o", [B, 128, NBLK, 512], F32, kind="ExternalOutput")
    st = nc.dram_tensor("st", [128, 896], FP16, kind="ExternalInput")
    kc = nc.dram_tensor("kc", [B, S, D], F32, kind="ExternalOutput")
    vc = nc.dram_tensor("vc", [B, S, D], F32, kind="ExternalOutput")

    with tile.TileContext(nc) as tc:
        with (
            tc.tile_pool(name="consts", bufs=1) as consts,
            tc.tile_pool(name="big", bufs=2) as big,
            tc.tile_pool(name="pt", bufs=6) as ptp,
            tc.tile_pool(name="osb", bufs=6) as obp,
            tc.tile_pool(name="nrm", bufs=4) as nrm,
            tc.tile_pool(name="st", bufs=2, space="PSUM") as stp,
            tc.tile_pool(name="op", bufs=2, space="PSUM") as opp,
        ):
            # Mask on the (otherwise idle) SWDGE ring so it lands in
            # parallel with the sync-ring input stream.
            mk_t = consts.tile([128, 512], FP16, tag="mk")
            nc.gpsimd.dma_start(out=mk_t, in_=mk[:, :])

            # All input loads for BOTH units plus the cache copies are
            # emitted up front on the sync ring: none of them have compute
            # dependencies, so the SP sequencer streams them back to back
            # from t=0. Output stores are emitted later (also on sync) and
            # their data-waits resolve just in time.
            kt_cs, qt_cs, va_cs = [], [], []
            starters = {}
            for u in range(B):
                kt_c = [None, None]  # [128,1024]: t-tiles 8i..8i+7
                qt_c = [None] * 4  # [128,2048]: blocks 4i..4i+3
                va_c = [None, None]  # [128,2048]: t-tiles 8i..8i+7
                kt_cs.append(kt_c)
                qt_cs.append(qt_c)
                va_cs.append(va_c)

                def _load(kind, i, u=u, kt_c=kt_c, qt_c=qt_c, va_c=va_c):
                    if kind == "kt":
                        t = big.tile(
                            [128, 1024], FP16, tag=f"kt{i}", name=f"kt{u}_{i}"
                        )
                        nc.sync.dma_start(
                            out=t, in_=kt[u, :, i * 1024 : (i + 1) * 1024]
                        )
                        kt_c[i] = t
                    elif kind == "qt":
                        t = big.tile(
                            [128, 2048], FP16, tag=f"qt{i}", name=f"qt{u}_{i}"
                        )
                        nc.sync.dma_start(
                            out=t, in_=qt[u, :, i * 2048 : (i + 1) * 2048]
                        )
                        qt_c[i] = t
                    else:
                        t = big.tile(
                            [128, 8 * VCOLS], FP16, tag=f"va{i}", name=f"va{u}_{i}"
                        )
                        nc.sync.dma_start(
                            out=t, in_=va[u, :, i * 8 * VCOLS : (i + 1) * 8 * VCOLS]
                        )
                        va_c[i] = t

                if u == 0:
                    # One packed starter tile covers block 0 (t-tile 0 only)
                    # so the first matmul issues a few microseconds in.
                    st_t = big.tile([128, 896], FP16, tag="st", name="st_t")
                    nc.scalar.dma_start(out=st_t, in_=st[:, :])
                    starters[0] = (
                        st_t[:, 0:128], st_t[:, 128:640], st_t[:, 640:896],
                    )
                    order = [
                        ("kt", 0), ("qt", 0), ("va", 0), ("qt", 1),
                        ("va", 1), ("qt", 2), ("kt", 1), ("qt", 3),
                    ]
                else:
                    # Unit 1 runs its blocks in descending order, so the
                    # tail-needed tiles load first .. (everything is resident
                    # long before unit 1 starts anyway).
                    order = [
                        ("qt", 3), ("kt", 1), ("va", 1), ("va", 0),
                        ("kt", 0), ("qt", 2), ("qt", 1), ("qt", 0),
                    ]
                for kind, i in order:
                    _load(kind, i)

            # KV-cache store (slot_mapping is arange -> token-order copy);
            # DRAM->DRAM right behind the input loads, well before the
            # output stores need the ring.
            for u in range(B):
                nc.sync.dma_start(out=kc[u], in_=knat[u])
                nc.sync.dma_start(out=vc[u], in_=vnat[u])

            # One group = up to 2 t-tiles sharing an exp (FD=1024). The PV
            # matmuls for group g are emitted AFTER group g+1's QK matmuls
            # (lag-1 software pipeline) so the TensorE never sits behind the
            # exp on the critical path.
            pending = [None]  # (blk_done, o_ps, p_t, blk, t0, gsz, n_t, va_c)
            o_sb_cur = [None, 0]  # current 4-block out staging tile, #filled

            def _emit_mm2_and_maybe_norm(u):
                if pending[0] is None:
                    return
                (is_last, o_ps, p_t, blk, t0, gsz, n_t, rhs_list) = pending[0]
                pending[0] = None
                for j in range(gsz):
                    tt = t0 + j
                    for c in range(4):
                        # Two accumulation chunks share a PSUM bank; only the
                        # first may use start=True (start clears has_written
                        # bits of the WHOLE bank). The second chunk's first
                        # write lands on cleared bits and overwrites, so
                        # start=False always is correct.
                        nc.tensor.matmul(
                            o_ps[c // 2][:, (c % 2) * 129 : (c % 2) * 129 + 129],
                            lhsT=p_t[
                                :, j * 512 + c * 128 : j * 512 + (c + 1) * 128
                            ],
                            rhs=rhs_list[j],
                            start=(tt == 0 and c % 2 == 0),
                            stop=(tt == n_t - 1),
                            skip_group_check=(c % 2 == 1),
                        )
                if not is_last:
                    return
                sums = nrm.tile([128, 4], F32, tag="sums", name="sums")
                rec = nrm.tile([128, 4], F32, tag="rec", name="rec")
                for i in range(2):
                    src = o_ps[i].rearrange("p (a b) -> p a b", a=2)[:, :, 128]
                    nc.vector.tensor_copy(sums[:, 2 * i : 2 * i + 2], src)
                nc.vector.reciprocal(rec, sums)
                if u == 1 and blk < 4:
                    # The kernel's last four (small) blocks: store each
                    # immediately so the final transfer trailing the last
                    # compute is 256KB, not a 1MB batch.
                    o1 = obp.tile([128, 512], F32, tag="osb1", name="o_sb1")
                    for c in range(4):
                        nc.vector.tensor_scalar_mul(
                            o1[:, c * 128 : (c + 1) * 128],
                            o_ps[c // 2][:, (c % 2) * 129 : (c % 2) * 129 + 128],
                            rec[:, c : c + 1],
                        )
                    nc.gpsimd.dma_start(
                        out=o[u, :, blk : blk + 1, :],
                        in_=o1.rearrange("p (k f) -> p k f", k=1),
                    )
                    return
                # Normalized blocks stage into a 4-block batch tile; one DMA
                # per batch gives 8KB descriptors and fewer SWDGE ops.
                if o_sb_cur[0] is None:
                    o_sb_cur[0] = obp.tile([128, 2048], F32, tag="osb", name="o_sb")
                    o_sb_cur[1] = 0
                o_sb = o_sb_cur[0]
                off = (blk % 4) * 512
                for c in range(4):
                    nc.vector.tensor_scalar_mul(
                        o_sb[:, off + c * 128 : off + (c + 1) * 128],
                        o_ps[c // 2][:, (c % 2) * 129 : (c % 2) * 129 + 128],
                        rec[:, c : c + 1],
                    )
                o_sb_cur[1] += 1
                if o_sb_cur[1] == 4:
                    b0 = (blk // 4) * 4
                    nc.gpsimd.dma_start(
                        out=o[u, :, b0 : b0 + 4, :],
                        in_=o_sb.rearrange("p (k f) -> p k f", k=4),
                    )
                    o_sb_cur[0] = None

            for u in range(B):
                kt_c, qt_c, va_c = kt_cs[u], qt_cs[u], va_cs[u]
                if u in starters:
                    kt_st, qt_st, va_st = starters[u]

                blk_order = range(NBLK) if u == 0 else reversed(range(NBLK))
                for blk in blk_order:
                    n_t = blk + 1
                    starter = u == 0 and blk == 0
                    if starter:
                        rhs_q = qt_st
                    else:
                        rhs_q = qt_c[blk // 4][
                            :, (blk % 4) * 512 : (blk % 4 + 1) * 512
                        ]
                    o_ps = [
                        opp.tile([128, 258], F32, tag=f"o{i}", name=f"o_ps{i}")
                        for i in range(2)
                    ]
                    for t0 in range(0, n_t, 2):
                        gsz = min(2, n_t - t0)
                        s_ps = stp.tile([128, 1024], F32, tag="s", name="s_ps")
                        for j in range(gsz):
                            tt = t0 + j
                            sl = s_ps[:, j * 512 : (j + 1) * 512]
                            nc.tensor.matmul(
                                sl,
                                lhsT=(
                                    kt_st
                                    if starter
                                    else kt_c[tt // 8][
                                        :, (tt % 8) * 128 : (tt % 8 + 1) * 128
                                    ]
                                ),
                                rhs=rhs_q,
                                start=True,
                                stop=True,
                            )
                        p_t = ptp.tile([128, 1024], FP16, tag="p", name="p_t")
                        nc.scalar.activation(
                            p_t[:, : gsz * 512],
                            s_ps[:, : gsz * 512],
                            mybir.ActivationFunctionType.Exp,
                            scale=SCALE,
                        )
                        if t0 + gsz == n_t:
                            # Diagonal t-tile (always the last of the block):
                            # zero the upper-triangular part after exp.
                            dsl = p_t[:, (gsz - 1) * 512 : gsz * 512]
                            nc.vector.tensor_mul(dsl, dsl, mk_t)
                        rhs_list = [
                            (
                                va_st[:, 0:129]
                                if starter
                                else va_c[(t0 + j) // 8][
                                    :,
                                    ((t0 + j) % 8) * VCOLS : ((t0 + j) % 8) * VCOLS
                                    + 129,
                                ]
                            )
                            for j in range(gsz)
                        ]
                        _emit_mm2_and_maybe_norm(u)
                        pending[0] = (
                            t0 + gsz == n_t, o_ps, p_t, blk, t0, gsz, n_t, rhs_list,
                        )
                _emit_mm2_and_maybe_norm(u)

    nc.compile()
    return nc


def _get_nc():
    global _NC
    if _NC is None:
        _NC = _build_nc()
    return _NC


def _prepare_in_maps(q, k, v):
    q = np.asarray(q, dtype=np.float32)
    k = np.asarray(k, dtype=np.float32)
    v = np.asarray(v, dtype=np.float32)

    # qt[kv, b, d, blk, g, s_in]
    q6 = q.reshape(B, NBLK, 128, HKV, G, D)
    qt = np.ascontiguousarray(
        np.transpose(q6, (3, 0, 5, 1, 4, 2)).astype(np.float16)
    ).reshape(HKV, B, 128, NBLK * 512)

    # kt[kv, b, d, t]
    k4 = k.reshape(B, S, HKV, D)
    kt = np.ascontiguousarray(np.transpose(k4, (2, 0, 3, 1)).astype(np.float16))

    # va[kv, b, t_in, tt, col]: col<128 = v, col 128 = 1.0, rest 0
    v5 = v.reshape(B, NBLK, 128, HKV, D)
    va = np.zeros((HKV, B, 128, NBLK, VCOLS), dtype=np.float16)
    va[..., :D] = np.transpose(v5, (3, 0, 2, 1, 4))
    va[..., D] = 1.0
    va = va.reshape(HKV, B, 128, NBLK * VCOLS)

    knat = np.ascontiguousarray(np.transpose(k4, (2, 0, 1, 3)))
    vnat = np.ascontiguousarray(
        np.transpose(v.reshape(B, S, HKV, D), (2, 0, 1, 3))
    )

    idx = np.arange(128)
    mk = np.tile(
        (idx[:, None] <= idx[None, :]).astype(np.float16), (1, G)
    )  # [t_in, (g, s_in)] = t_in <= s_in

    in_maps = []
    for c in range(NCORES):
        starter = np.concatenate(
            [kt[c, 0, :, 0:128], qt[c, 0, :, 0:512], va[c, 0, :, 0:VCOLS]], axis=1
        )
        in_maps.append(
            {
                "qt": qt[c],
                "kt": kt[c],
                "va": va[c],
                "knat": knat[c],
                "vnat": vnat[c],
                "mk": mk,
                "st": np.ascontiguousarray(starter),
            }
        )
    return in_maps


def _assemble(results, k_cache, v_cache, slot_mapping):
    o_dev = np.stack([results[c]["o"] for c in range(NCORES)])
    o_dev = o_dev.reshape(HKV, B, 128, NBLK, G, D)  # [kv, b, s_in, blk, g, d]
    out = np.ascontiguousarray(
        np.transpose(o_dev, (1, 3, 2, 0, 4, 5))
    ).reshape(N, H, D)

    slots = np.asarray(slot_mapping)
    kcf = np.array(np.asarray(k_cache), dtype=np.float32, copy=True)
    vcf = np.array(np.asarray(v_cache), dtype=np.float32, copy=True)
    kc_dev = np.stack([results[c]["kc"] for c in range(NCORES)])  # [kv, b, t, d]
    vc_dev = np.stack([results[c]["vc"] for c in range(NCORES)])
    kc_rows = np.transpose(kc_dev, (1, 2, 0, 3)).reshape(N, HKV * D)
    vc_rows = np.transpose(vc_dev, (1, 2, 0, 3)).reshape(N, HKV * D)
    kcf[slots] = kc_rows
    vcf[slots] = vc_rows
    return out, kcf, vcf


def _run(q, k, v, k_cache, v_cache, slot_mapping, seq_len, trace=False):
    assert int(seq_len) == S
    nc = _get_nc()
    in_maps = _prepare_in_maps(q, k, v)
    res = run_bass_kernel_spmd(nc, in_maps, core_ids=list(range(NCORES)), trace=trace)
    out, kcf, vcf = _assemble(res.results, k_cache, v_cache, slot_mapping)
    return (out, kcf, vcf), res


def kernel(q, k, v, k_cache, v_cache, slot_mapping, seq_len):
    (out, kcf, vcf), _ = _run(q, k, v, k_cache, v_cache, slot_mapping, seq_len)
    return out, kcf, vcf
